# revision 23
# baseline (speedup 1.0000x reference)
"""Trainium2 Bass kernel for coverage-weighted additive (Bahdanau) attention.

Reference computation (per row b of B=16384):
    proj_key[k] = Ua @ topics[b,k]                      (K=8 topics, HID=EMB=512)
    q_i = query[b] @ (Wa.T)^i                           (i = 1..K, sequential re-projection)
    s_i = va . tanh(q_i + proj_key[i]) + va_b
    alphas = softmax(s * coverage[b])                   (over K)
    mt[b] = sum_k alphas[k] * topics[b,k]

Sharding: data-parallel over B across 8 NeuronCores (2048 rows each); weights
replicated. The Wa power matrices are precomputed on host (float64) so all K
query projections become independent matmuls.

Per-core kernel layout (batch-major): for each 128-row tile, query/topic tiles
are PE-transposed so the contraction dim lands on partitions, then each topic's
tanh argument (qn_i + pk_i) is accumulated in one PSUM bank by 8 chained
matmuls. ScalarE applies tanh, VectorE does a fused multiply-reduce against va
for the score, softmax runs batch-major on [128, 8] tiles, and the weighted
topic sum uses per-partition ScalarE scaling plus VectorE adds.
"""

import sys

if "/opt/trn_rl_repo" not in sys.path:
    sys.path.insert(0, "/opt/trn_rl_repo")

import os

import numpy as np

import concourse.bass as bass
import concourse.mybir as mybir
import concourse.tile as tile
import concourse.bass_utils as _bu
from concourse.bass_utils import run_bass_kernel_spmd

# Allow walrus to dedupe/overlap LDWEIGHTS (A/B via TRN_LDWOPT=0).
if not getattr(_bu, "_ldwopt_patched", False):
    _orig_run_command = _bu.run_command

    def _run_command(argv, **kw):
        if os.environ.get("TRN_LDWOPT", "0") == "1" and isinstance(argv, list):
            argv = [a.replace("--enable-ldw-opt=false", "--enable-ldw-opt=true")
                    if isinstance(a, str) else a for a in argv]
        return _orig_run_command(argv, **kw)

    _bu.run_command = _run_command
    _bu._ldwopt_patched = True

B, K, HID, EMB = 16384, 8, 512, 512
NCORES = 8
BC = B // NCORES          # rows per core
P = 128                   # partitions
NBT = BC // P             # 128-row tiles per core
KT = HID // P             # contraction subtiles (4)

F32 = mybir.dt.float32


def split_excess_waits(nc, max_waits=1):
    """This walrus build rejects >few sync waits per instruction; move extras
    onto preceding NoOps on the same engine (semantically identical)."""
    for f in nc.m.functions:
        for bb in f.blocks:
            insts = bb.instructions
            i = 0
            while i < len(insts):
                inst = insts[i]
                si = inst.sync_info
                if si is not None and si.on_wait and len(si.on_wait) > max_waits:
                    waits = list(si.on_wait)
                    keep = waits[-max_waits:]
                    extra = waits[:-max_waits]
                    for j in range(0, len(extra), max_waits):
                        nop = mybir.InstNoOp(
                            name=f"{inst.name}-wsplit{j}", ins=[], outs=[])
                        nop.engine = inst.engine
                        nop.sync_info = mybir.SyncInfo(
                            on_wait=extra[j:j + max_waits], on_update=[])
                        insts.insert(i, nop)
                        i += 1
                    inst.sync_info = mybir.SyncInfo(
                        on_wait=keep, on_update=list(si.on_update or []))
                i += 1


def build_nc(mm_dt=F32):
    """Build the per-core Bass module. mm_dt: dtype used for the qn/pk matmuls
    (float32 or float32r via bitcast views; data is always stored as f32)."""
    nc = bass.Bass(num_swdge_queues=4)

    BF16 = mybir.dt.bfloat16
    q_d = nc.dram_tensor("q", [BC, HID], F32, kind="ExternalInput")
    top_d = nc.dram_tensor("topics", [BC, K, EMB], F32, kind="ExternalInput")
    qhi_d = nc.dram_tensor("qhi", [BC, HID], BF16, kind="ExternalInput")
    qlo_d = nc.dram_tensor("qlo", [BC, HID], BF16, kind="ExternalInput")
    thi_d = nc.dram_tensor("thi", [BC, K, EMB], BF16, kind="ExternalInput")
    tlo_d = nc.dram_tensor("tlo", [BC, K, EMB], BF16, kind="ExternalInput")
    cov_d = nc.dram_tensor("cov", [BC, K], F32, kind="ExternalInput")
    mall_d = nc.dram_tensor("mall", [HID, K * HID], F32, kind="ExternalInput")
    uat_d = nc.dram_tensor("uat", [EMB, HID], F32, kind="ExternalInput")
    varep_d = nc.dram_tensor("varep", [P, HID], F32, kind="ExternalInput")
    vab_d = nc.dram_tensor("vab", [P, 1], F32, kind="ExternalInput")
    ident_d = nc.dram_tensor("ident", [P, P], F32, kind="ExternalInput")

    mt_d = nc.dram_tensor("mt", [BC, EMB], F32, kind="ExternalOutput")
    alphas_d = nc.dram_tensor("alphas", [BC, K], F32, kind="ExternalOutput")

    with tile.TileContext(nc) as tc:
        with (
            tc.tile_pool(name="consts", bufs=1) as consts,
            tc.tile_pool(name="stage", bufs=2) as stage,
            tc.tile_pool(name="bigH", bufs=2) as bigH,
            tc.tile_pool(name="bigT", bufs=2) as bigT,
            tc.tile_pool(name="bigF", bufs=2) as bigF,
            tc.tile_pool(name="work", bufs=2) as work,
            tc.tile_pool(name="small", bufs=2) as small,
            tc.tile_pool(name="junkp", bufs=1) as junkp,
            tc.tile_pool(name="ps_tr", bufs=3, space="PSUM") as ps_tr,
            tc.tile_pool(name="ps_arg", bufs=5, space="PSUM") as ps_arg,
        ):
            wdma = nc.sync.dma_start if mm_dt == F32 else nc.gpsimd.dma_start
            mall_sb = consts.tile([P, KT, K * HID], mm_dt)
            uat_sb = consts.tile([P, KT, HID], mm_dt)
            mall_r = mall_d.rearrange("(ko p) n -> p ko n", p=P)
            uat_r = uat_d.rearrange("(ko p) n -> p ko n", p=P)

            def load_weights(phase):
                if mm_dt == F32:
                    if phase == 0:
                        nc.sync.dma_start(mall_sb[:], mall_r)
                        nc.sync.dma_start(uat_sb[:], uat_r)
                    return
                if phase == 0:      # uat first (pk of every group needs it)
                    for kt in range(KT):
                        wdma(uat_sb[:, kt, :], uat_r[:, kt, :])
                    for kt in range(KT):
                        for ih in range(2):
                            sl = slice(ih * 2 * HID, (ih + 1) * 2 * HID)
                            wdma(mall_sb[:, kt, sl], mall_r[:, kt, sl])
                else:               # topics 4-7 projections can land later
                    for kt in range(KT):
                        for ih in range(2, 4):
                            sl = slice(ih * 2 * HID, (ih + 1) * 2 * HID)
                            wdma(mall_sb[:, kt, sl], mall_r[:, kt, sl])
            varep_sb = consts.tile([P, HID], F32)
            nc.sync.dma_start(varep_sb[:], varep_d[:])
            vab_sb = consts.tile([P, 1], F32)
            nc.sync.dma_start(vab_sb[:], vab_d[:])
            ident_sb = consts.tile([P, P], mm_dt)
            wdma(ident_sb[:], ident_d[:])
            ident_mm = ident_sb[:]
            identb_sb = consts.tile([P, P], mybir.dt.bfloat16)
            nc.gpsimd.dma_start(identb_sb[:], ident_d[:])

            for bt in range(NBT):
                rows = slice(bt * P, (bt + 1) * P)

                qhiT = work.tile([P, KT, P], mybir.dt.bfloat16, tag="qhiT")
                nc.sync.dma_start_transpose(
                    qhiT[:], qhi_d[rows, :].rearrange("b (po pi) -> b po pi",
                                                      pi=P))
                qloT = work.tile([P, KT, P], mybir.dt.bfloat16, tag="qloT")
                nc.sync.dma_start_transpose(
                    qloT[:], qlo_d[rows, :].rearrange("b (po pi) -> b po pi",
                                                      pi=P))
                thiT = bigH.tile([P, K, KT, P], mybir.dt.bfloat16, tag="thiT")
                tloT = bigH.tile([P, K, KT, P], mybir.dt.bfloat16, tag="tloT")
                for k in range(K):
                    nc.sync.dma_start_transpose(
                        thiT[:, k, :, :],
                        thi_d[rows, k, :].rearrange("b (po pi) -> b po pi",
                                                    pi=P))
                    nc.sync.dma_start_transpose(
                        tloT[:, k, :, :],
                        tlo_d[rows, k, :].rearrange("b (po pi) -> b po pi",
                                                    pi=P))
                top_f32 = bigF.tile([P, K, EMB], F32, tag="top_f32")
                nc.sync.dma_start(top_f32[:], top_d[rows, :, :])
                if bt <= 1:
                    load_weights(bt)
                cov_t = small.tile([P, K], F32, tag="cov")
                nc.sync.dma_start(cov_t[:], cov_d[rows, :])

                # Reconstruct transposed f32r tiles from XBAR-transposed
                # bf16 hi/lo pairs: psum = I.T @ hi + I.T @ lo, then the ACT
                # copy rounds to f32r.
                qT = work.tile([P, KT, P], mm_dt, tag="qT")
                pst = ps_tr.tile([P, KT, P], F32, tag="pst")
                nc.tensor.matmul(pst[:].rearrange("p a b -> p (a b)"),
                                 identb_sb[:],
                                 qhiT[:].rearrange("p a b -> p (a b)"),
                                 start=True, stop=False)
                nc.tensor.matmul(pst[:].rearrange("p a b -> p (a b)"),
                                 identb_sb[:],
                                 qloT[:].rearrange("p a b -> p (a b)"),
                                 start=False, stop=True)
                nc.scalar.copy(qT[:], pst[:])

                topT = bigT.tile([P, K, KT, P], mm_dt, tag="topT")
                for k in range(K):
                    pst = ps_tr.tile([P, KT, P], F32, tag="pst")
                    nc.tensor.matmul(pst[:].rearrange("p a b -> p (a b)"),
                                     identb_sb[:],
                                     thiT[:, k].rearrange("p a b -> p (a b)"),
                                     start=True, stop=False)
                    nc.tensor.matmul(pst[:].rearrange("p a b -> p (a b)"),
                                     identb_sb[:],
                                     tloT[:, k].rearrange("p a b -> p (a b)"),
                                     start=False, stop=True)
                    nc.scalar.copy(topT[:, k, :, :], pst[:])

                scores = small.tile([P, K], F32, tag="scores")
                for g in range(2):          # topic groups of 4
                    args = [ps_arg.tile([P, HID], F32, tag="arg",
                                        name=f"arg_{bt}_{g}_{ii}")
                            for ii in range(4)]
                    # qn: same qT stationary reused across the 4 topics
                    for kt in range(KT):
                        for ii in range(4):
                            i = g * 4 + ii
                            nc.tensor.matmul(
                                args[ii][:],
                                qT[:, kt, :],
                                mall_sb[:, kt, i * HID:(i + 1) * HID],
                                start=(kt == 0), stop=False)
                    for ii in range(4):     # pk_i = topics_i @ Ua.T
                        i = g * 4 + ii
                        for et in range(KT):
                            nc.tensor.matmul(
                                args[ii][:],
                                topT[:, i, et, :],
                                uat_sb[:, et, :],
                                start=False, stop=(et == KT - 1))
                        tanh_t = work.tile([P, HID], F32, tag="tanh")
                        nc.scalar.activation(
                            tanh_t[:], args[ii][:],
                            mybir.ActivationFunctionType.Tanh)
                        junk = junkp.tile([P, HID], F32, tag="junk")
                        nc.vector.tensor_tensor(
                            junk[:], tanh_t[:], varep_sb[:],
                            mybir.AluOpType.mult)
                        nc.vector.tensor_reduce(
                            scores[:, i:i + 1], junk[:],
                            mybir.AxisListType.X, mybir.AluOpType.add)

                # softmax over K with coverage weighting, batch-major
                nc.vector.tensor_scalar_add(
                    scores[:], scores[:], vab_sb[:, 0:1])
                sc2 = small.tile([P, K], F32, tag="sc2")
                nc.vector.tensor_tensor(
                    sc2[:], scores[:], cov_t[:], mybir.AluOpType.mult)
                mx = small.tile([P, 1], F32, tag="mx")
                nc.vector.tensor_reduce(
                    mx[:], sc2[:], mybir.AxisListType.X, mybir.AluOpType.max)
                nmx = small.tile([P, 1], F32, tag="nmx")
                nc.vector.tensor_scalar_mul(nmx[:], mx[:], -1.0)
                ex = small.tile([P, K], F32, tag="ex")
                ssum = small.tile([P, 1], F32, tag="ssum")
                nc.scalar.activation(
                    ex[:], sc2[:], mybir.ActivationFunctionType.Exp,
                    bias=nmx[:, 0:1], scale=1.0, accum_out=ssum[:, 0:1])
                rs = small.tile([P, 1], F32, tag="rs")
                nc.vector.reciprocal(rs[:], ssum[:])
                alpha = small.tile([P, K], F32, tag="alpha")
                nc.vector.tensor_scalar_mul(alpha[:], ex[:], rs[:, 0:1])
                nc.sync.dma_start(alphas_d[rows, :], alpha[:])

                # mt = sum_k alpha_k * topics_k  (ScalarE per-partition scale,
                # VectorE accumulate)
                acc = work.tile([P, EMB], F32, tag="acc")
                nc.scalar.mul(acc[:], top_f32[:, 0, :], alpha[:, 0:1])
                for k in range(1, K):
                    prod = work.tile([P, EMB], F32, tag="prod")
                    nc.scalar.mul(prod[:], top_f32[:, k, :], alpha[:, k:k + 1])
                    nc.vector.tensor_add(acc[:], acc[:], prod[:])
                nc.sync.dma_start(mt_d[rows, :], acc[:])

    split_excess_waits(nc)
    return nc


def host_inputs(query, topics, coverage_vector, Ua_w, Wa_w, va_w, va_b):
    """Host-side precompute + per-core sharding."""
    WaT = Wa_w.astype(np.float64).T
    mats = []
    M = np.eye(HID, dtype=np.float64)
    for _ in range(K):
        M = M @ WaT
        mats.append(M)
    mall = np.concatenate(mats, axis=1).astype(np.float32)   # [HID, K*HID]
    uat = np.ascontiguousarray(Ua_w.T).astype(np.float32)    # [EMB, HID]
    varep = np.ascontiguousarray(
        np.broadcast_to(va_w.astype(np.float32), (P, HID)))
    vab = np.full((P, 1), np.float32(va_b[0]), dtype=np.float32)
    ident = np.eye(P, dtype=np.float32)

    import ml_dtypes
    bf16 = ml_dtypes.bfloat16
    q32 = query.astype(np.float32)
    t32 = topics.astype(np.float32)
    qhi = q32.astype(bf16)
    qlo = (q32 - qhi.astype(np.float32)).astype(bf16)
    thi = t32.astype(bf16)
    tlo = (t32 - thi.astype(np.float32)).astype(bf16)

    in_maps = []
    for c in range(NCORES):
        r = slice(c * BC, (c + 1) * BC)
        in_maps.append({
            "q": np.ascontiguousarray(q32[r]),
            "topics": np.ascontiguousarray(t32[r]),
            "qhi": np.ascontiguousarray(qhi[r]),
            "qlo": np.ascontiguousarray(qlo[r]),
            "thi": np.ascontiguousarray(thi[r]),
            "tlo": np.ascontiguousarray(tlo[r]),
            "cov": np.ascontiguousarray(coverage_vector[r]).astype(np.float32),
            "mall": mall,
            "uat": uat,
            "varep": varep,
            "vab": vab,
            "ident": ident,
        })
    return in_maps


_NC_CACHE = {}


def run(query, topics, coverage_vector, Ua_w, Wa_w, va_w, va_b,
        mm_dt=F32, trace=False):
    key = str(mm_dt)
    if key not in _NC_CACHE:
        _NC_CACHE[key] = build_nc(mm_dt)
    nc = _NC_CACHE[key]
    in_maps = host_inputs(query, topics, coverage_vector, Ua_w, Wa_w,
                          va_w, va_b)
    res = run_bass_kernel_spmd(nc, in_maps, core_ids=list(range(NCORES)),
                               trace=trace)
    mt = np.concatenate([res.results[c]["mt"] for c in range(NCORES)], axis=0)
    alphas = np.concatenate(
        [res.results[c]["alphas"] for c in range(NCORES)], axis=0)
    return (mt, alphas), res


def kernel(query, topics, coverage_vector, Ua_w, Wa_w, va_w, va_b):
    (mt, alphas), _ = run(np.asarray(query), np.asarray(topics),
                          np.asarray(coverage_vector), np.asarray(Ua_w),
                          np.asarray(Wa_w), np.asarray(va_w),
                          np.asarray(va_b))
    return mt, alphas


# revision 24
# speedup vs baseline: 2.1800x; 2.1800x over previous
"""Trainium2 Bass kernel for coverage-weighted additive (Bahdanau) attention.

Reference computation (per row b of B=16384):
    proj_key[k] = Ua @ topics[b,k]                      (K=8 topics, HID=EMB=512)
    q_i = query[b] @ (Wa.T)^i                           (i = 1..K, sequential re-projection)
    s_i = va . tanh(q_i + proj_key[i]) + va_b
    alphas = softmax(s * coverage[b])                   (over K)
    mt[b] = sum_k alphas[k] * topics[b,k]

Sharding: data-parallel over B across 8 NeuronCores (2048 rows each); weights
replicated. The Wa power matrices are precomputed on host (float64) so all K
query projections become independent matmuls.

Per-core kernel layout (batch-major): for each 128-row tile, query/topic tiles
are PE-transposed so the contraction dim lands on partitions, then each topic's
tanh argument (qn_i + pk_i) is accumulated in one PSUM bank by 8 chained
matmuls. ScalarE applies tanh, VectorE does a fused multiply-reduce against va
for the score, softmax runs batch-major on [128, 8] tiles, and the weighted
topic sum uses per-partition ScalarE scaling plus VectorE adds.
"""

import sys

if "/opt/trn_rl_repo" not in sys.path:
    sys.path.insert(0, "/opt/trn_rl_repo")

import os

import numpy as np

import concourse.bass as bass
import concourse.mybir as mybir
import concourse.tile as tile
import concourse.bass_utils as _bu
from concourse.bass_utils import run_bass_kernel_spmd

# Allow walrus to dedupe/overlap LDWEIGHTS (A/B via TRN_LDWOPT=0).
if not getattr(_bu, "_ldwopt_patched", False):
    _orig_run_command = _bu.run_command

    def _run_command(argv, **kw):
        if os.environ.get("TRN_LDWOPT", "0") == "1" and isinstance(argv, list):
            argv = [a.replace("--enable-ldw-opt=false", "--enable-ldw-opt=true")
                    if isinstance(a, str) else a for a in argv]
        return _orig_run_command(argv, **kw)

    _bu.run_command = _run_command
    _bu._ldwopt_patched = True

B, K, HID, EMB = 16384, 8, 512, 512
NCORES = 8
BC = B // NCORES          # rows per core
P = 128                   # partitions
NBT = BC // P             # 128-row tiles per core
KT = HID // P             # contraction subtiles (4)

F32 = mybir.dt.float32


def split_excess_waits(nc, max_waits=1):
    """This walrus build rejects >few sync waits per instruction; move extras
    onto preceding NoOps on the same engine (semantically identical)."""
    for f in nc.m.functions:
        for bb in f.blocks:
            insts = bb.instructions
            i = 0
            while i < len(insts):
                inst = insts[i]
                si = inst.sync_info
                if si is not None and si.on_wait and len(si.on_wait) > max_waits:
                    waits = list(si.on_wait)
                    keep = waits[-max_waits:]
                    extra = waits[:-max_waits]
                    for j in range(0, len(extra), max_waits):
                        nop = mybir.InstNoOp(
                            name=f"{inst.name}-wsplit{j}", ins=[], outs=[])
                        nop.engine = inst.engine
                        nop.sync_info = mybir.SyncInfo(
                            on_wait=extra[j:j + max_waits], on_update=[])
                        insts.insert(i, nop)
                        i += 1
                    inst.sync_info = mybir.SyncInfo(
                        on_wait=keep, on_update=list(si.on_update or []))
                i += 1


def build_nc(mm_dt=F32):
    """Build the per-core Bass module. mm_dt: dtype used for the qn/pk matmuls
    (float32 or float32r via bitcast views; data is always stored as f32)."""
    nc = bass.Bass(num_swdge_queues=4)

    q_d = nc.dram_tensor("q", [BC, HID], F32, kind="ExternalInput")
    top_d = nc.dram_tensor("topics", [BC, K, EMB], F32, kind="ExternalInput")
    cov_d = nc.dram_tensor("cov", [BC, K], F32, kind="ExternalInput")
    mall_d = nc.dram_tensor("mall", [HID, K * HID], F32, kind="ExternalInput")
    uat_d = nc.dram_tensor("uat", [EMB, HID], F32, kind="ExternalInput")
    varep_d = nc.dram_tensor("varep", [P, HID], F32, kind="ExternalInput")
    vab_d = nc.dram_tensor("vab", [P, 1], F32, kind="ExternalInput")
    ident_d = nc.dram_tensor("ident", [P, P], F32, kind="ExternalInput")

    mt_d = nc.dram_tensor("mt", [BC, EMB], F32, kind="ExternalOutput")
    alphas_d = nc.dram_tensor("alphas", [BC, K], F32, kind="ExternalOutput")

    with tile.TileContext(nc) as tc:
        with (
            tc.tile_pool(name="consts", bufs=1) as consts,
            tc.tile_pool(name="stage", bufs=2) as stage,
            tc.tile_pool(name="big", bufs=2) as big,
            tc.tile_pool(name="bigT", bufs=2) as bigT,
            tc.tile_pool(name="bigF", bufs=2) as bigF,
            tc.tile_pool(name="work", bufs=2) as work,
            tc.tile_pool(name="small", bufs=2) as small,
            tc.tile_pool(name="junkp", bufs=1) as junkp,
            tc.tile_pool(name="ps_tr", bufs=3, space="PSUM") as ps_tr,
            tc.tile_pool(name="ps_arg", bufs=5, space="PSUM") as ps_arg,
        ):
            wdma = nc.sync.dma_start if mm_dt == F32 else nc.gpsimd.dma_start
            mall_sb = consts.tile([P, KT, K * HID], mm_dt)
            uat_sb = consts.tile([P, KT, HID], mm_dt)
            mall_r = mall_d.rearrange("(ko p) n -> p ko n", p=P)
            uat_r = uat_d.rearrange("(ko p) n -> p ko n", p=P)

            def load_weights(phase):
                if mm_dt == F32:
                    if phase == 0:
                        nc.sync.dma_start(mall_sb[:], mall_r)
                        nc.sync.dma_start(uat_sb[:], uat_r)
                    return
                if phase == 0:      # uat first (pk of every group needs it)
                    for kt in range(KT):
                        wdma(uat_sb[:, kt, :], uat_r[:, kt, :])
                    for kt in range(KT):
                        for ih in range(2):
                            sl = slice(ih * 2 * HID, (ih + 1) * 2 * HID)
                            wdma(mall_sb[:, kt, sl], mall_r[:, kt, sl])
                else:               # topics 4-7 projections can land later
                    for kt in range(KT):
                        for ih in range(2, 4):
                            sl = slice(ih * 2 * HID, (ih + 1) * 2 * HID)
                            wdma(mall_sb[:, kt, sl], mall_r[:, kt, sl])
            varep_sb = consts.tile([P, HID], F32)
            nc.sync.dma_start(varep_sb[:], varep_d[:])
            vab_sb = consts.tile([P, 1], F32)
            nc.sync.dma_start(vab_sb[:], vab_d[:])
            ident_sb = consts.tile([P, P], mm_dt)
            wdma(ident_sb[:], ident_d[:])
            ident_mm = ident_sb[:]

            for bt in range(NBT):
                rows = slice(bt * P, (bt + 1) * P)

                q_nat = work.tile([P, HID], mm_dt)
                wdma(q_nat[:], q_d[rows, :])
                top_nat = big.tile([P, K, EMB], mm_dt, tag="top_nat")
                wdma(top_nat[:], top_d[rows, :, :])
                top_f32 = bigF.tile([P, K, EMB], F32, tag="top_f32")
                nc.sync.dma_start(top_f32[:], top_d[rows, :, :])
                if bt <= 1:
                    load_weights(bt)
                cov_t = small.tile([P, K], F32, tag="cov")
                nc.sync.dma_start(cov_t[:], cov_d[rows, :])

                # Transpose q tile: [128b, 512h'] -> qT [128h'-part, kt, 128b].
                # The 4 blocks share one PSUM bank so a single wide ACT copy
                # moves them to SBUF.
                qT = work.tile([P, KT, P], mm_dt, tag="qT")
                pst = ps_tr.tile([P, KT, P], mm_dt, tag="pst")
                for kt in range(KT):
                    nc.tensor.matmul(
                        pst[:, kt, :], q_nat[:, kt * P:(kt + 1) * P],
                        ident_mm, is_transpose=True,
                        start=(kt == 0), stop=(kt == KT - 1))
                nc.scalar.copy(qT[:], pst[:])

                # Transpose topic tiles: topT[e-part, topic, et, 128b]
                topT = bigT.tile([P, K, KT, P], mm_dt, tag="topT")
                for k in range(K):
                    pst = ps_tr.tile([P, KT, P], mm_dt, tag="pst")
                    for et in range(KT):
                        nc.tensor.matmul(
                            pst[:, et, :],
                            top_nat[:, k, et * P:(et + 1) * P],
                            ident_mm, is_transpose=True,
                            start=(et == 0), stop=(et == KT - 1))
                    nc.scalar.copy(topT[:, k, :, :], pst[:])

                scores = small.tile([P, K], F32, tag="scores")
                for g in range(2):          # topic groups of 4
                    args = [ps_arg.tile([P, HID], F32, tag="arg",
                                        name=f"arg_{bt}_{g}_{ii}")
                            for ii in range(4)]
                    # qn: same qT stationary reused across the 4 topics
                    for kt in range(KT):
                        for ii in range(4):
                            i = g * 4 + ii
                            nc.tensor.matmul(
                                args[ii][:],
                                qT[:, kt, :],
                                mall_sb[:, kt, i * HID:(i + 1) * HID],
                                start=(kt == 0), stop=False)
                    for ii in range(4):     # pk_i = topics_i @ Ua.T
                        i = g * 4 + ii
                        for et in range(KT):
                            nc.tensor.matmul(
                                args[ii][:],
                                topT[:, i, et, :],
                                uat_sb[:, et, :],
                                start=False, stop=(et == KT - 1))
                        tanh_t = work.tile([P, HID], F32, tag="tanh")
                        nc.scalar.activation(
                            tanh_t[:], args[ii][:],
                            mybir.ActivationFunctionType.Tanh)
                        junk = junkp.tile([P, HID], F32, tag="junk")
                        nc.vector.tensor_tensor(
                            junk[:], tanh_t[:], varep_sb[:],
                            mybir.AluOpType.mult)
                        nc.vector.tensor_reduce(
                            scores[:, i:i + 1], junk[:],
                            mybir.AxisListType.X, mybir.AluOpType.add)

                # softmax over K with coverage weighting, batch-major
                nc.vector.tensor_scalar_add(
                    scores[:], scores[:], vab_sb[:, 0:1])
                sc2 = small.tile([P, K], F32, tag="sc2")
                nc.vector.tensor_tensor(
                    sc2[:], scores[:], cov_t[:], mybir.AluOpType.mult)
                mx = small.tile([P, 1], F32, tag="mx")
                nc.vector.tensor_reduce(
                    mx[:], sc2[:], mybir.AxisListType.X, mybir.AluOpType.max)
                nmx = small.tile([P, 1], F32, tag="nmx")
                nc.vector.tensor_scalar_mul(nmx[:], mx[:], -1.0)
                ex = small.tile([P, K], F32, tag="ex")
                ssum = small.tile([P, 1], F32, tag="ssum")
                nc.scalar.activation(
                    ex[:], sc2[:], mybir.ActivationFunctionType.Exp,
                    bias=nmx[:, 0:1], scale=1.0, accum_out=ssum[:, 0:1])
                rs = small.tile([P, 1], F32, tag="rs")
                nc.vector.reciprocal(rs[:], ssum[:])
                alpha = small.tile([P, K], F32, tag="alpha")
                nc.vector.tensor_scalar_mul(alpha[:], ex[:], rs[:, 0:1])
                nc.sync.dma_start(alphas_d[rows, :], alpha[:])

                # mt = sum_k alpha_k * topics_k  (ScalarE per-partition scale,
                # VectorE accumulate)
                acc = work.tile([P, EMB], F32, tag="acc")
                nc.scalar.mul(acc[:], top_f32[:, 0, :], alpha[:, 0:1])
                for k in range(1, K):
                    prod = work.tile([P, EMB], F32, tag="prod")
                    nc.scalar.mul(prod[:], top_f32[:, k, :], alpha[:, k:k + 1])
                    nc.vector.tensor_add(acc[:], acc[:], prod[:])
                nc.sync.dma_start(mt_d[rows, :], acc[:])

    split_excess_waits(nc)
    return nc


def host_inputs(query, topics, coverage_vector, Ua_w, Wa_w, va_w, va_b):
    """Host-side precompute + per-core sharding."""
    WaT = Wa_w.astype(np.float64).T
    mats = []
    M = np.eye(HID, dtype=np.float64)
    for _ in range(K):
        M = M @ WaT
        mats.append(M)
    mall = np.concatenate(mats, axis=1).astype(np.float32)   # [HID, K*HID]
    uat = np.ascontiguousarray(Ua_w.T).astype(np.float32)    # [EMB, HID]
    varep = np.ascontiguousarray(
        np.broadcast_to(va_w.astype(np.float32), (P, HID)))
    vab = np.full((P, 1), np.float32(va_b[0]), dtype=np.float32)
    ident = np.eye(P, dtype=np.float32)

    in_maps = []
    for c in range(NCORES):
        r = slice(c * BC, (c + 1) * BC)
        in_maps.append({
            "q": np.ascontiguousarray(query[r]).astype(np.float32),
            "topics": np.ascontiguousarray(topics[r]).astype(np.float32),
            "cov": np.ascontiguousarray(coverage_vector[r]).astype(np.float32),
            "mall": mall,
            "uat": uat,
            "varep": varep,
            "vab": vab,
            "ident": ident,
        })
    return in_maps


_NC_CACHE = {}


def run(query, topics, coverage_vector, Ua_w, Wa_w, va_w, va_b,
        mm_dt=F32, trace=False):
    key = str(mm_dt)
    if key not in _NC_CACHE:
        _NC_CACHE[key] = build_nc(mm_dt)
    nc = _NC_CACHE[key]
    in_maps = host_inputs(query, topics, coverage_vector, Ua_w, Wa_w,
                          va_w, va_b)
    res = run_bass_kernel_spmd(nc, in_maps, core_ids=list(range(NCORES)),
                               trace=trace)
    mt = np.concatenate([res.results[c]["mt"] for c in range(NCORES)], axis=0)
    alphas = np.concatenate(
        [res.results[c]["alphas"] for c in range(NCORES)], axis=0)
    return (mt, alphas), res


def kernel(query, topics, coverage_vector, Ua_w, Wa_w, va_w, va_b):
    (mt, alphas), _ = run(np.asarray(query), np.asarray(topics),
                          np.asarray(coverage_vector), np.asarray(Ua_w),
                          np.asarray(Wa_w), np.asarray(va_w),
                          np.asarray(va_b))
    return mt, alphas


# revision 25
# speedup vs baseline: 2.3291x; 1.0684x over previous
"""Trainium2 Bass kernel for coverage-weighted additive (Bahdanau) attention.

Reference computation (per row b of B=16384):
    proj_key[k] = Ua @ topics[b,k]                      (K=8 topics, HID=EMB=512)
    q_i = query[b] @ (Wa.T)^i                           (i = 1..K, sequential re-projection)
    s_i = va . tanh(q_i + proj_key[i]) + va_b
    alphas = softmax(s * coverage[b])                   (over K)
    mt[b] = sum_k alphas[k] * topics[b,k]

Sharding: data-parallel over B across 8 NeuronCores (2048 rows each); weights
replicated. The Wa power matrices are precomputed on host (float64) so all K
query projections become independent matmuls.

Per-core kernel layout (batch-major): for each 128-row tile, query/topic tiles
are PE-transposed so the contraction dim lands on partitions, then each topic's
tanh argument (qn_i + pk_i) is accumulated in one PSUM bank by 8 chained
matmuls. ScalarE applies tanh, VectorE does a fused multiply-reduce against va
for the score, softmax runs batch-major on [128, 8] tiles, and the weighted
topic sum uses per-partition ScalarE scaling plus VectorE adds.
"""

import sys

if "/opt/trn_rl_repo" not in sys.path:
    sys.path.insert(0, "/opt/trn_rl_repo")

import os

import numpy as np

import concourse.bass as bass
import concourse.mybir as mybir
import concourse.tile as tile
import concourse.bass_utils as _bu
from concourse.bass_utils import run_bass_kernel_spmd

# Allow walrus to dedupe/overlap LDWEIGHTS (A/B via TRN_LDWOPT=0).
if not getattr(_bu, "_ldwopt_patched", False):
    _orig_run_command = _bu.run_command

    def _run_command(argv, **kw):
        if os.environ.get("TRN_LDWOPT", "1") == "1" and isinstance(argv, list):
            argv = [a.replace("--enable-ldw-opt=false", "--enable-ldw-opt=true")
                    if isinstance(a, str) else a for a in argv]
        return _orig_run_command(argv, **kw)

    _bu.run_command = _run_command
    _bu._ldwopt_patched = True

B, K, HID, EMB = 16384, 8, 512, 512
NCORES = 8
BC = B // NCORES          # rows per core
P = 128                   # partitions
NBT = BC // P             # 128-row tiles per core
KT = HID // P             # contraction subtiles (4)

F32 = mybir.dt.float32


def split_excess_waits(nc, max_waits=1):
    """This walrus build rejects >few sync waits per instruction; move extras
    onto preceding NoOps on the same engine (semantically identical)."""
    for f in nc.m.functions:
        for bb in f.blocks:
            insts = bb.instructions
            i = 0
            while i < len(insts):
                inst = insts[i]
                si = inst.sync_info
                if si is not None and si.on_wait and len(si.on_wait) > max_waits:
                    waits = list(si.on_wait)
                    keep = waits[-max_waits:]
                    extra = waits[:-max_waits]
                    for j in range(0, len(extra), max_waits):
                        nop = mybir.InstNoOp(
                            name=f"{inst.name}-wsplit{j}", ins=[], outs=[])
                        nop.engine = inst.engine
                        nop.sync_info = mybir.SyncInfo(
                            on_wait=extra[j:j + max_waits], on_update=[])
                        insts.insert(i, nop)
                        i += 1
                    inst.sync_info = mybir.SyncInfo(
                        on_wait=keep, on_update=list(si.on_update or []))
                i += 1


def build_nc(mm_dt=F32):
    """Build the per-core Bass module. mm_dt: dtype used for the qn/pk matmuls
    (float32 or float32r via bitcast views; data is always stored as f32)."""
    nc = bass.Bass(num_swdge_queues=4)

    q_d = nc.dram_tensor("q", [BC, HID], F32, kind="ExternalInput")
    top_d = nc.dram_tensor("topics", [BC, K, EMB], F32, kind="ExternalInput")
    cov_d = nc.dram_tensor("cov", [BC, K], F32, kind="ExternalInput")
    mall_d = nc.dram_tensor("mall", [HID, K * HID], F32, kind="ExternalInput")
    uat_d = nc.dram_tensor("uat", [EMB, HID], F32, kind="ExternalInput")
    varep_d = nc.dram_tensor("varep", [P, HID], F32, kind="ExternalInput")
    vab_d = nc.dram_tensor("vab", [P, 1], F32, kind="ExternalInput")
    ident_d = nc.dram_tensor("ident", [P, P], F32, kind="ExternalInput")

    mt_d = nc.dram_tensor("mt", [BC, EMB], F32, kind="ExternalOutput")
    alphas_d = nc.dram_tensor("alphas", [BC, K], F32, kind="ExternalOutput")

    with tile.TileContext(nc) as tc:
        with (
            tc.tile_pool(name="consts", bufs=1) as consts,
            tc.tile_pool(name="stage", bufs=2) as stage,
            tc.tile_pool(name="big", bufs=3) as big,
            tc.tile_pool(name="bigT", bufs=2) as bigT,
            tc.tile_pool(name="work", bufs=2) as work,
            tc.tile_pool(name="small", bufs=2) as small,
            tc.tile_pool(name="junkp", bufs=1) as junkp,
            tc.tile_pool(name="ps_tr", bufs=3, space="PSUM") as ps_tr,
            tc.tile_pool(name="ps_arg", bufs=5, space="PSUM") as ps_arg,
        ):
            wdma = nc.sync.dma_start if mm_dt == F32 else nc.gpsimd.dma_start
            mall_sb = consts.tile([P, KT, K * HID], mm_dt)
            uat_sb = consts.tile([P, KT, HID], mm_dt)
            mall_r = mall_d.rearrange("(ko p) n -> p ko n", p=P)
            uat_r = uat_d.rearrange("(ko p) n -> p ko n", p=P)

            def load_weights(phase):
                if mm_dt == F32:
                    if phase == 0:
                        nc.sync.dma_start(mall_sb[:], mall_r)
                        nc.sync.dma_start(uat_sb[:], uat_r)
                    return
                if phase == 0:      # uat first (pk of every group needs it)
                    for kt in range(KT):
                        wdma(uat_sb[:, kt, :], uat_r[:, kt, :])
                    for kt in range(KT):
                        for ih in range(2):
                            sl = slice(ih * 2 * HID, (ih + 1) * 2 * HID)
                            wdma(mall_sb[:, kt, sl], mall_r[:, kt, sl])
                else:               # topics 4-7 projections can land later
                    for kt in range(KT):
                        for ih in range(2, 4):
                            sl = slice(ih * 2 * HID, (ih + 1) * 2 * HID)
                            wdma(mall_sb[:, kt, sl], mall_r[:, kt, sl])
            varep_sb = consts.tile([P, HID], F32)
            nc.sync.dma_start(varep_sb[:], varep_d[:])
            vab_sb = consts.tile([P, 1], F32)
            nc.sync.dma_start(vab_sb[:], vab_d[:])
            ident_sb = consts.tile([P, P], mm_dt)
            wdma(ident_sb[:], ident_d[:])
            ident_mm = ident_sb[:]

            for bt in range(NBT):
                rows = slice(bt * P, (bt + 1) * P)

                q_nat = work.tile([P, HID], mm_dt)
                wdma(q_nat[:], q_d[rows, :])
                top_nat = big.tile([P, K, EMB], mm_dt, tag="top_nat")
                wdma(top_nat[:], top_d[rows, :, :])
                if bt <= 1:
                    load_weights(bt)
                cov_t = small.tile([P, K], F32, tag="cov")
                nc.sync.dma_start(cov_t[:], cov_d[rows, :])

                # Transpose q tile: [128b, 512h'] -> qT [128h'-part, kt, 128b].
                # The 4 blocks share one PSUM bank so a single wide ACT copy
                # moves them to SBUF.
                qT = work.tile([P, KT, P], mm_dt, tag="qT")
                pst = ps_tr.tile([P, KT, P], mm_dt, tag="pst")
                for kt in range(KT):
                    nc.tensor.matmul(
                        pst[:, kt, :], q_nat[:, kt * P:(kt + 1) * P],
                        ident_mm, is_transpose=True,
                        start=(kt == 0), stop=(kt == KT - 1))
                nc.scalar.copy(qT[:], pst[:])

                # Transpose topic tiles: topT[e-part, topic, et, 128b]
                topT = bigT.tile([P, K, KT, P], mm_dt, tag="topT")
                for k in range(K):
                    pst = ps_tr.tile([P, KT, P], mm_dt, tag="pst")
                    for et in range(KT):
                        nc.tensor.matmul(
                            pst[:, et, :],
                            top_nat[:, k, et * P:(et + 1) * P],
                            ident_mm, is_transpose=True,
                            start=(et == 0), stop=(et == KT - 1))
                    nc.scalar.copy(topT[:, k, :, :], pst[:])

                scores = small.tile([P, K], F32, tag="scores")
                for g in range(2):          # topic groups of 4
                    args = [ps_arg.tile([P, HID], F32, tag="arg",
                                        name=f"arg_{bt}_{g}_{ii}")
                            for ii in range(4)]
                    # qn: same qT stationary reused across the 4 topics
                    for kt in range(KT):
                        for ii in range(4):
                            i = g * 4 + ii
                            nc.tensor.matmul(
                                args[ii][:],
                                qT[:, kt, :],
                                mall_sb[:, kt, i * HID:(i + 1) * HID],
                                start=(kt == 0), stop=False)
                    for ii in range(4):     # pk_i = topics_i @ Ua.T
                        i = g * 4 + ii
                        for et in range(KT):
                            nc.tensor.matmul(
                                args[ii][:],
                                topT[:, i, et, :],
                                uat_sb[:, et, :],
                                start=False, stop=(et == KT - 1))
                        tanh_t = work.tile([P, HID], F32, tag="tanh")
                        nc.scalar.activation(
                            tanh_t[:], args[ii][:],
                            mybir.ActivationFunctionType.Tanh)
                        junk = junkp.tile([P, HID], F32, tag="junk")
                        nc.vector.tensor_tensor(
                            junk[:], tanh_t[:], varep_sb[:],
                            mybir.AluOpType.mult)
                        nc.vector.tensor_reduce(
                            scores[:, i:i + 1], junk[:],
                            mybir.AxisListType.X, mybir.AluOpType.add)

                # softmax over K with coverage weighting, batch-major
                nc.vector.tensor_scalar_add(
                    scores[:], scores[:], vab_sb[:, 0:1])
                sc2 = small.tile([P, K], F32, tag="sc2")
                nc.vector.tensor_tensor(
                    sc2[:], scores[:], cov_t[:], mybir.AluOpType.mult)
                mx = small.tile([P, 1], F32, tag="mx")
                nc.vector.tensor_reduce(
                    mx[:], sc2[:], mybir.AxisListType.X, mybir.AluOpType.max)
                nmx = small.tile([P, 1], F32, tag="nmx")
                nc.vector.tensor_scalar_mul(nmx[:], mx[:], -1.0)
                ex = small.tile([P, K], F32, tag="ex")
                ssum = small.tile([P, 1], F32, tag="ssum")
                nc.scalar.activation(
                    ex[:], sc2[:], mybir.ActivationFunctionType.Exp,
                    bias=nmx[:, 0:1], scale=1.0, accum_out=ssum[:, 0:1])
                rs = small.tile([P, 1], F32, tag="rs")
                nc.vector.reciprocal(rs[:], ssum[:])
                alpha = small.tile([P, K], F32, tag="alpha")
                nc.vector.tensor_scalar_mul(alpha[:], ex[:], rs[:, 0:1])
                nc.sync.dma_start(alphas_d[rows, :], alpha[:])

                # mt = sum_k alpha_k * topics_k  (ScalarE per-partition scale,
                # VectorE accumulate)
                acc = work.tile([P, EMB], F32, tag="acc")
                nc.scalar.mul(acc[:], top_nat[:, 0, :], alpha[:, 0:1])
                for k in range(1, K):
                    prod = work.tile([P, EMB], F32, tag="prod")
                    nc.scalar.mul(prod[:], top_nat[:, k, :], alpha[:, k:k + 1])
                    nc.vector.tensor_add(acc[:], acc[:], prod[:])
                nc.sync.dma_start(mt_d[rows, :], acc[:])

    split_excess_waits(nc)
    return nc


def host_inputs(query, topics, coverage_vector, Ua_w, Wa_w, va_w, va_b):
    """Host-side precompute + per-core sharding."""
    WaT = Wa_w.astype(np.float64).T
    mats = []
    M = np.eye(HID, dtype=np.float64)
    for _ in range(K):
        M = M @ WaT
        mats.append(M)
    mall = np.concatenate(mats, axis=1).astype(np.float32)   # [HID, K*HID]
    uat = np.ascontiguousarray(Ua_w.T).astype(np.float32)    # [EMB, HID]
    varep = np.ascontiguousarray(
        np.broadcast_to(va_w.astype(np.float32), (P, HID)))
    vab = np.full((P, 1), np.float32(va_b[0]), dtype=np.float32)
    ident = np.eye(P, dtype=np.float32)

    in_maps = []
    for c in range(NCORES):
        r = slice(c * BC, (c + 1) * BC)
        in_maps.append({
            "q": np.ascontiguousarray(query[r]).astype(np.float32),
            "topics": np.ascontiguousarray(topics[r]).astype(np.float32),
            "cov": np.ascontiguousarray(coverage_vector[r]).astype(np.float32),
            "mall": mall,
            "uat": uat,
            "varep": varep,
            "vab": vab,
            "ident": ident,
        })
    return in_maps


_NC_CACHE = {}


def run(query, topics, coverage_vector, Ua_w, Wa_w, va_w, va_b,
        mm_dt=F32, trace=False):
    key = str(mm_dt)
    if key not in _NC_CACHE:
        _NC_CACHE[key] = build_nc(mm_dt)
    nc = _NC_CACHE[key]
    in_maps = host_inputs(query, topics, coverage_vector, Ua_w, Wa_w,
                          va_w, va_b)
    res = run_bass_kernel_spmd(nc, in_maps, core_ids=list(range(NCORES)),
                               trace=trace)
    mt = np.concatenate([res.results[c]["mt"] for c in range(NCORES)], axis=0)
    alphas = np.concatenate(
        [res.results[c]["alphas"] for c in range(NCORES)], axis=0)
    return (mt, alphas), res


def kernel(query, topics, coverage_vector, Ua_w, Wa_w, va_w, va_b):
    (mt, alphas), _ = run(np.asarray(query), np.asarray(topics),
                          np.asarray(coverage_vector), np.asarray(Ua_w),
                          np.asarray(Wa_w), np.asarray(va_w),
                          np.asarray(va_b))
    return mt, alphas
